# revision 11
# baseline (speedup 1.0000x reference)
"""Trainium2 Bass kernel for nn_DWT_Layer: 3-level 2D db4 DWT (symmetric mode).

Input  x: (16, 3, 1024, 1024) fp32.
Output:   (16, 3, 64, 128, 128) fp32 — the flattened/truncated wavelet pyramid
          [cA3, cH3, cV3, cD3, cH2, cV2, cD2, cH1, cV1, cD1(truncated)].

Sharding: pure data parallel — the 48 (batch*channel) images are split 6 per
NeuronCore across 8 cores; no communication.

v2 design (vs fp32 baseline):
  * all intermediates fp16: matmuls run 1 cyc/row (4x over fp32), and the
    width-pass MACs on DVE hit the 4x_2p perf mode (needs 2-byte dtype +
    unit-stride operands in SBUF).
  * width pass is POLYPHASE: the symmetric-extension buffer is split into
    even/odd column buffers so every tap reads unit-stride.  ext_e[u]=ext[2u],
    ext_o[u]=ext[2u+1]; out[c] = sum_k frev[2k]*ext_e[c+k] + frev[2k+1]*ext_o[c+k].
  * height pass: banded fp16 matmuls; the hi filter block sits at a
    128-aligned row offset so psum->staging copies are slot-aligned.
  * detail outputs staged per whole section in fp16 and written with a few
    large DMAs (HWDGE instruction overhead is ~650ns each); host converts the
    fp16 output back to fp32.
"""
import numpy as np

# ----------------------------------------------------------------- constants
DEC_LO = np.array([-0.010597401784997278, 0.032883011666982945,
                   0.030841381835986965, -0.18703481171888114,
                   -0.027983769416983849, 0.63088076792959036,
                   0.71484657055254153, 0.23037781330885523], dtype=np.float64)
L = 8
DEC_HI = np.array([(-1.0) ** (k + 1) * DEC_LO[L - 1 - k] for k in range(L)],
                  dtype=np.float64)
FREV_LO = [float(v) for v in DEC_LO[::-1].astype(np.float32)]
FREV_HI = [float(v) for v in DEC_HI[::-1].astype(np.float32)]
TAPS_ARR = np.tile(np.array(FREV_LO + FREV_HI, dtype=np.float32)[None, :],
                   (128, 1))  # unused; kept for test.py compat

B, C, H, W = 16, 3, 1024, 1024
N_CORES = 8
IMGS_PER_CORE = 6
IMG_ELEMS = H * W

# (N, Np, S_in, n_det_slots)
LEVELS = [
    (1024, 515, 8, 5),
    (515, 261, 5, 3),
    (261, 134, 3, 2),
]

SECT = {}
_cur = 0
for _name, _n in [("cA3", 134), ("cH3", 134), ("cV3", 134), ("cD3", 134),
                  ("cH2", 261), ("cV2", 261), ("cD2", 261),
                  ("cH1", 515), ("cV1", 515), ("cD1", 515)]:
    SECT[_name] = (_cur, _n)
    _cur += _n * _n
CD1_FULL_ROWS = 469
CD1_PART_COLS = 404
assert SECT["cD1"][0] + CD1_FULL_ROWS * 515 + CD1_PART_COLS == IMG_ELEMS


def nprime(N):
    return (N + 5) // 2 + 1


def ext_index(j, N):
    if j < 6:
        return 5 - j
    if j < N + 6:
        return j - 6
    return 2 * N + 5 - j


def dwt_matrix(N, filt):
    Np = nprime(N)
    M = np.zeros((Np, N), dtype=np.float64)
    filtrev = filt[::-1]
    for i in range(Np):
        for t in range(L):
            M[i, ext_index(2 * i + t, N)] += filtrev[t]
    return M


def hi_off(Np):
    """row offset of the hi section: 128-aligned so hi t-tiles map 1:1 to
    128-row detail slots (and engine APs start at partition 0)."""
    return ((Np + 127) // 128) * 128


def stacked_matrix(N):
    Np = nprime(N)
    off = hi_off(Np)
    M2 = np.zeros((off + Np, N), dtype=np.float64)
    M2[0:Np] = dwt_matrix(N, DEC_LO)
    M2[off:] = dwt_matrix(N, DEC_HI)
    return M2.astype(np.float16)


def band_blocks(N):
    M2 = stacked_matrix(N)
    R = M2.shape[0]
    kt = (N + 127) // 128
    ot = (R + 127) // 128
    per_t = []
    for t in range(ot):
        qs = []
        for q in range(kt):
            blk = M2[t * 128:(t + 1) * 128, q * 128:(q + 1) * 128]
            if np.any(blk != 0):
                qs.append(q)
        per_t.append(qs)
    return per_t, kt, ot, R


def const_weights(N):
    """packed lhsT blocks [128, nblocks, 128] fp16 + index map {(t,q): b}."""
    M2 = stacked_matrix(N)
    per_t, kt, ot, R = band_blocks(N)
    blocks = [(t, q) for t in range(ot) for q in per_t[t]]
    arr = np.zeros((128, len(blocks), 128), dtype=np.float16)
    idx = {}
    for b, (t, q) in enumerate(blocks):
        blk = M2[t * 128:(t + 1) * 128, q * 128:(q + 1) * 128]  # [mt, kq]
        arr[:blk.shape[1], b, :blk.shape[0]] = blk.T
        idx[(t, q)] = b
    return arr, idx, per_t


WC = {N: const_weights(N) for N, _, _, _ in LEVELS}


# polyphase geometry per level: interior sizes and buffer width
def poly_geom(N):
    Np = nprime(N)
    ne = (N + 1) // 2
    no = N // 2
    U = Np + 3
    return Np, ne, no, U


# mirror copies (dst_buf, dst0, n, src_buf, src_hi) meaning
#   dst[dst0 : dst0+n] = src[src_hi : src_hi-n : -1]
# computed from the generic plan; verified in proto.py against pywt semantics.
def mirror_copies(N):
    Np, ne, no, U = poly_geom(N)

    def src_loc(xi):
        if xi % 2 == 0:
            return 0, xi // 2 + 3
        return 1, (xi - 1) // 2 + 3

    out = []
    for buf, n_int in ((0, ne), (1, no)):
        for rng in (range(0, 3), range(3 + n_int, U)):
            runs = [(u, src_loc(ext_index(2 * u + buf, N))) for u in rng]
            i = 0
            while i < len(runs):
                du0, (sb, su0) = runs[i]
                j = i + 1
                while (j < len(runs) and runs[j][1][0] == sb
                       and runs[j][0] == runs[j - 1][0] + 1
                       and runs[j][1][1] == runs[j - 1][1][1] - 1):
                    j += 1
                out.append((buf, du0, j - i, sb, su0))
                i = j
    return out


# ---- engine assignment knobs ----
DEINT_EVEN_ENG = "scalar"   # fp32->fp16 de-interleave, even phase
DEINT_ODD_ENG = "gpsimd"
MIRROR_ENG = "gpsimd"
AA_ENG = "scalar"           # psum -> next-level ext copies
DET_ENG = {"cH": "scalar", "cV": "gpsimd", "cD": "gpsimd", "cA": "scalar"}
MAC_SLOT_SPLIT = {1: 2, 2: 1, 3: 1}  # ops per tap (split over slots)

XF_BUFS = 2
EXT1_BUFS = 2
WB1_BUFS = 2
DET_BUFS = 2

_BUILT = None


def _eng(nc, name):
    return getattr(nc, name)


def _copy(nc, eng, out, in_):
    """engine-dispatched copy: ACT uses activation-Copy, others tensor_copy."""
    if eng == "scalar":
        nc.scalar.copy(out=out, in_=in_)
    else:
        getattr(nc, eng).tensor_copy(out=out, in_=in_)


def _emit_mirrors(nc, N, ee, eo, S):
    bufs = {0: ee, 1: eo}
    for (db, du0, n, sb, su0) in mirror_copies(N):
        _copy(nc, MIRROR_ENG, bufs[db][:, 0:S, du0:du0 + n],
              bufs[sb][:, 0:S, su0:su0 - n:-1])


def _emit_macs(nc, tmp_pool, lvl, N, ee, eo, wb, S):
    """wb[:, s, fi*Np + c] = sum_t frev[t] * ext[2c+t] via polyphase.

    The fused scalar_tensor_tensor runs at 1 elem/cycle on DVE (no perf
    modes), so each tap is a tensor_scalar mult (4x_2p, 0.25 cyc/elem) into a
    ping-pong tmp followed by a tensor_tensor add (2x_1p, 0.5 cyc/elem):
    0.69 cyc/elem amortized instead of 1."""
    import concourse.mybir as mybir
    Np = nprime(N)
    nsub = MAC_SLOT_SPLIT[lvl]
    bounds = [round(S * i / nsub) for i in range(nsub + 1)]
    f16 = mybir.dt.float16
    for si in range(nsub):
        s0, s1 = bounds[si], bounds[si + 1]
        ns = s1 - s0
        for k in range(4):
            for fi, frev in ((0, FREV_LO), (1, FREV_HI)):
                base = fi * Np
                for ph, buf in ((0, ee), (1, eo)):
                    t = 2 * k + ph
                    src = buf[:, s0:s1, k:k + Np]
                    dst = wb[:, s0:s1, base:base + Np]
                    if t == 0:
                        nc.vector.tensor_scalar_mul(dst, src, frev[t])
                    else:
                        tmp = tmp_pool.tile([128, ns, Np], f16,
                                            tag=f"tmp{lvl}_{si}", bufs=3,
                                            name=f"tmp{lvl}_{si}_{fi}_{t}")
                        nc.vector.tensor_scalar_mul(tmp[:], src, frev[t])
                        nc.vector.tensor_tensor(
                            out=dst, in0=dst, in1=tmp[:],
                            op=mybir.AluOpType.add)


def build_bass(n_images=IMGS_PER_CORE, repeats=1):
    import concourse.mybir as mybir
    import concourse.tile as tile
    from concourse import bacc
    from contextlib import ExitStack

    nc = bacc.Bacc("TRN2", target_bir_lowering=False, debug=False)

    xin = nc.dram_tensor("xin", (n_images, H, W), mybir.dt.float32,
                         kind="ExternalInput").ap()
    out = nc.dram_tensor("out", (n_images, IMG_ELEMS), mybir.dt.float16,
                         kind="ExternalOutput").ap()
    wdram = {}
    for N, _, _, _ in LEVELS:
        arr, _, _ = WC[N]
        wdram[N] = nc.dram_tensor(f"w{N}", arr.shape, mybir.dt.float16,
                                  kind="ExternalInput").ap()

    with tile.TileContext(nc) as tc, ExitStack() as ctx:
        cpool = ctx.enter_context(tc.tile_pool(name="consts", bufs=1))
        extp = ctx.enter_context(tc.tile_pool(name="ext", bufs=1))
        wbp = ctx.enter_context(tc.tile_pool(name="wb", bufs=1))
        psp = ctx.enter_context(tc.tile_pool(name="ps", bufs=1, space="PSUM"))
        detp = ctx.enter_context(tc.tile_pool(name="det", bufs=1))

        wsb = {}
        for N, _, _, _ in LEVELS:
            arr, _, _ = WC[N]
            wsb[N] = cpool.tile(list(arr.shape), mybir.dt.float16,
                                name=f"wsb{N}")
            nc.sync.dma_start(out=wsb[N][:], in_=wdram[N])

        for _rep in range(repeats):
            for img in range(n_images):
                _emit_image(nc, tc, extp, wbp, psp, detp, wsb, xin, out, img)

    nc.compile()
    return nc


def _emit_image(nc, tc, extp, wbp, psp, detp, wsb, xin, out, img):
    import concourse.mybir as mybir
    f16 = mybir.dt.float16

    # ---------------- L1: load + de-interleave + MACs ----------------
    N1, P1 = 1024, 515
    _, ne1, no1, U1 = poly_geom(N1)
    ee1 = extp.tile([128, 8, U1], f16, tag="ext1e", bufs=EXT1_BUFS,
                    name=f"ee1_{img}")
    eo1 = extp.tile([128, 8, U1], f16, tag="ext1o", bufs=EXT1_BUFS,
                    name=f"eo1_{img}")
    for h in range(2):
        xf = extp.tile([128, 4, W], mybir.dt.float32, tag="xf", bufs=XF_BUFS,
                       name=f"xf_{img}_{h}")
        src = xin[img, 512 * h:512 * (h + 1), :].rearrange(
            "(s p) w -> p s w", p=128)
        nc.sync.dma_start(out=xf[:], in_=src)
        _copy(nc, DEINT_EVEN_ENG, ee1[:, 4 * h:4 * h + 4, 3:3 + ne1],
              xf[:, :, 0:W:2])
        _copy(nc, DEINT_ODD_ENG, eo1[:, 4 * h:4 * h + 4, 3:3 + no1],
              xf[:, :, 1:W:2])
    _emit_mirrors(nc, N1, ee1, eo1, 8)

    wb1 = wbp.tile([128, 8, 2 * P1], f16, tag="wb1", bufs=WB1_BUFS,
                   name=f"wb1_{img}")
    _emit_macs(nc, wbp, 1, N1, ee1, eo1, wb1, 8)

    # next-level aa staging (fp16, straight layout) + polyphase ext buffers.
    # The aa quadrant lands contiguously in aa2/aa3 (one ACT copy per psum
    # tile); Pool then de-interleaves SBUF->SBUF into the ext buffers.
    # Tail slots are memset so unwritten partitions stay finite.
    _, ne2, no2, U2 = poly_geom(515)
    aa2 = extp.tile([128, 5, 515], f16, tag="aa2", bufs=1, name=f"aa2_{img}")
    nc.gpsimd.memset(aa2[:, 4, :], 0.0)
    ee2 = extp.tile([128, 5, U2], f16, tag="ext2e", bufs=1, name=f"ee2_{img}")
    eo2 = extp.tile([128, 5, U2], f16, tag="ext2o", bufs=1, name=f"eo2_{img}")
    _, ne3, no3, U3 = poly_geom(261)
    aa3 = extp.tile([128, 3, 261], f16, tag="aa3", bufs=1, name=f"aa3_{img}")
    nc.gpsimd.memset(aa3[:, 2, :], 0.0)
    ee3 = extp.tile([128, 3, U3], f16, tag="ext3e", bufs=1, name=f"ee3_{img}")
    eo3 = extp.tile([128, 3, U3], f16, tag="ext3o", bufs=1, name=f"eo3_{img}")

    # detail staging tiles (fp16), whole sections, sec-major slots
    det1 = detp.tile([128, 10, 515], f16, tag="det1", bufs=DET_BUFS,
                     name=f"det1_{img}")  # cH1 slots 0-4, cV1 slots 5-9
    cd1 = detp.tile([128, 4, 515], f16, tag="cd1", bufs=DET_BUFS,
                    name=f"cd1_{img}")    # cD1 rows 0..511 (trunc at 469)
    det2 = detp.tile([128, 9, 261], f16, tag="det2", bufs=DET_BUFS,
                     name=f"det2_{img}")  # cH2 0-2, cV2 3-5, cD2 6-8
    det3 = detp.tile([128, 8, 134], f16, tag="det3", bufs=DET_BUFS,
                     name=f"det3_{img}")  # cA3 0-1, cH3 2-3, cV3 4-5, cD3 6-7

    def rhs1(q, c0, c1):
        return wb1[:, q, c0:c1]

    _emit_level_mm(nc, psp, wsb, img, N=1024, rhs=rhs1, aa=aa2,
                   det_cH=(det1, 0), det_cV=(det1, 5), det_cD=(cd1, 0),
                   det_cA=None, cd_trunc=True)
    # Pool de-interleaves aa2 -> polyphase ext (SBUF->SBUF; Pool can't
    # read PSUM), then mirrors
    nc.gpsimd.tensor_copy(out=ee2[:, 0:5, 3:3 + ne2], in_=aa2[:, :, 0:515:2])
    nc.gpsimd.tensor_copy(out=eo2[:, 0:5, 3:3 + no2], in_=aa2[:, :, 1:515:2])
    _emit_mirrors(nc, 515, ee2, eo2, 5)

    wb2 = wbp.tile([128, 5, 2 * 261], f16, tag="wb2", bufs=1,
                   name=f"wb2_{img}")
    _emit_macs(nc, wbp, 2, 515, ee2, eo2, wb2, 5)

    def rhs2(q, c0, c1):
        if q == 4:
            return wb2[0:3, 4, c0:c1]
        return wb2[:, q, c0:c1]

    _emit_level_mm(nc, psp, wsb, img, N=515, rhs=rhs2, aa=aa3,
                   det_cH=(det2, 0), det_cV=(det2, 3), det_cD=(det2, 6),
                   det_cA=None, cd_trunc=False)
    nc.gpsimd.tensor_copy(out=ee3[:, 0:3, 3:3 + ne3], in_=aa3[:, :, 0:261:2])
    nc.gpsimd.tensor_copy(out=eo3[:, 0:3, 3:3 + no3], in_=aa3[:, :, 1:261:2])
    _emit_mirrors(nc, 261, ee3, eo3, 3)

    wb3 = wbp.tile([128, 3, 2 * 134], f16, tag="wb3", bufs=1,
                   name=f"wb3_{img}")
    _emit_macs(nc, wbp, 3, 261, ee3, eo3, wb3, 3)

    def rhs3(q, c0, c1):
        if q == 2:
            return wb3[0:5, 2, c0:c1]
        return wb3[:, q, c0:c1]

    _emit_level_mm(nc, psp, wsb, img, N=261, rhs=rhs3,
                   aa=None,
                   det_cH=(det3, 2), det_cV=(det3, 4), det_cD=(det3, 6),
                   det_cA=(det3, 0), cd_trunc=False)

    # ---------------- output DMAs ----------------
    # L1: cH1+cV1 mains (4 full slots each), then a combined 3-row tail
    for sec, name in ((0, "cH1"), (1, "cV1")):
        b = SECT[name][0]
        dst = out[img, b:b + 512 * 515].rearrange("(s p w) -> p s w",
                                                  p=128, w=515)
        nc.sync.dma_start(out=dst, in_=det1[:, 5 * sec:5 * sec + 4, :])
    bh = SECT["cH1"][0]
    # combined tail: rows 512..514 of cH1 and cV1 via sec-strided AP
    dstT = out[img, bh:bh + 2 * 515 * 515].rearrange(
        "(sec p w) -> p sec w", sec=2, w=515)
    nc.sync.dma_start(out=dstT[512:515, :, :], in_=det1[0:3, 4:10:5, :])
    # cD1: 3 full slots, then 85 rows, then the 404-col partial row
    bd = SECT["cD1"][0]
    dst = out[img, bd:bd + 384 * 515].rearrange("(s p w) -> p s w",
                                                p=128, w=515)
    nc.sync.dma_start(out=dst, in_=cd1[:, 0:3, :])
    dst = out[img, bd + 384 * 515:bd + 469 * 515].rearrange(
        "(p w) -> p w", w=515)
    nc.sync.dma_start(out=dst, in_=cd1[0:85, 3, :])
    dst = out[img, bd + 469 * 515:bd + 469 * 515 + 404].rearrange(
        "(p w) -> p w", w=404)
    nc.sync.dma_start(out=dst, in_=cd1[85:86, 3, 0:404])
    # L2: three sections, contiguous: two 128-row passes + 5-row tail
    b2 = SECT["cH2"][0]
    dst2 = out[img, b2:b2 + 3 * 261 * 261].rearrange(
        "(sec p w) -> p sec w", sec=3, w=261)
    nc.sync.dma_start(out=dst2[0:128, :, :], in_=det2[:, 0:9:3, :])
    nc.sync.dma_start(out=dst2[128:256, :, :], in_=det2[:, 1:9:3, :])
    nc.sync.dma_start(out=dst2[256:261, :, :], in_=det2[0:5, 2:9:3, :])
    # L3: four sections, contiguous from offset 0: main + 6-row tail
    dst3 = out[img, 0:4 * 134 * 134].rearrange(
        "(sec p w) -> p sec w", sec=4, w=134)
    nc.sync.dma_start(out=dst3[0:128, :, :], in_=det3[:, 0:8:2, :])
    nc.sync.dma_start(out=dst3[128:134, :, :], in_=det3[0:6, 1:8:2, :])


def _free_chunks(Np):
    out = []
    for base in (0, Np):
        c = 0
        while c < Np:
            e = min(c + 512, Np)
            out.append((base + c, base + e))
            c = e
    return out


def _emit_level_mm(nc, psp, wsb, img, N, rhs, aa, det_cH, det_cV, det_cD,
                   det_cA, cd_trunc):
    """height-pass matmuls + one ACT psum->sbuf copy per (tile, quadrant).

    Each quadrant gets a [128, PSPAD] fp32 psum tile (PSPAD is a multiple of
    512 so every ring buffer stays bank-aligned); matmul column-groups of
    <=512 land in separate banks, and a single ACT copy drains the whole
    quadrant (engine reads may cross banks)."""
    import concourse.mybir as mybir

    Np = nprime(N)
    arr, idx, per_t = WC[N]
    OFF = hi_off(Np)
    R = OFF + Np
    ot = (R + 127) // 128
    lo_tiles = (Np + 127) // 128
    pspad = 1024 if Np > 512 else 512
    pstag = f"ps{pspad}"
    sub = [(c, min(c + 512, Np)) for c in range(0, Np, 512)]

    for t in range(ot):
        is_lo = t < lo_tiles
        slot = t if is_lo else t - OFF // 128
        mrows = min(128, Np - 128 * slot)
        last_hi = (not is_lo) and slot == lo_tiles - 1
        qs = per_t[t]
        if not qs:
            continue
        quadrants = (0,) if (cd_trunc and last_hi) else (0, 1)
        for qd in quadrants:
            ps = psp.tile([128, pspad], mybir.dt.float32, tag=pstag,
                          bufs=3 if pspad == 1024 else 2,
                          name=f"ps_{img}_{N}_{t}_{qd}")
            for (c0, c1) in sub:
                w = c1 - c0
                for ki, q in enumerate(qs):
                    kq = min(128, N - q * 128)
                    r = rhs(q, qd * Np + c0, qd * Np + c1)
                    nc.tensor.matmul(
                        ps[0:mrows, c0:c1],
                        wsb[N][0:kq, idx[(t, q)], 0:mrows],
                        r,
                        start=(ki == 0), stop=(ki == len(qs) - 1))
            # single drain copy for the whole quadrant
            if is_lo:
                if qd == 0 and aa is not None:
                    nc.scalar.copy(out=aa[0:mrows, slot, 0:Np],
                                   in_=ps[0:mrows, 0:Np])
                elif qd == 0:
                    dt_tile, s0 = det_cA
                    nc.scalar.copy(out=dt_tile[0:mrows, s0 + slot, 0:Np],
                                   in_=ps[0:mrows, 0:Np])
                else:
                    dt_tile, s0 = det_cV
                    nc.scalar.copy(out=dt_tile[0:mrows, s0 + slot, 0:Np],
                                   in_=ps[0:mrows, 0:Np])
            else:
                if qd == 0:
                    dt_tile, s0 = det_cH
                    nc.scalar.copy(out=dt_tile[0:mrows, s0 + slot, 0:Np],
                                   in_=ps[0:mrows, 0:Np])
                else:
                    dt_tile, s0 = det_cD
                    nc.scalar.copy(out=dt_tile[0:mrows, s0 + slot, 0:Np],
                                   in_=ps[0:mrows, 0:Np])


# ----------------------------------------------------------------- runner
def _get_built():
    global _BUILT
    if _BUILT is None:
        _BUILT = build_bass()
    return _BUILT


def kernel(x: np.ndarray) -> np.ndarray:
    from concourse import bass_utils

    x = np.ascontiguousarray(np.asarray(x), dtype=np.float32)
    assert x.shape == (B, C, H, W), x.shape
    nc = _get_built()

    imgs = x.reshape(B * C, H, W)
    in_maps = []
    for c in range(N_CORES):
        m = {"xin": imgs[c * IMGS_PER_CORE:(c + 1) * IMGS_PER_CORE]}
        for N, _, _, _ in LEVELS:
            m[f"w{N}"] = WC[N][0]
        in_maps.append(m)

    res = bass_utils.run_bass_kernel_spmd(nc, in_maps,
                                          core_ids=list(range(N_CORES)))
    outs = [res.results[c]["out"] for c in range(N_CORES)]
    flat = np.concatenate(outs, axis=0)  # [48, 1048576] fp16
    return flat.astype(np.float32).reshape(B, C, 64, 128, 128)


# revision 16
# speedup vs baseline: 2.3213x; 2.3213x over previous
"""Trainium2 Bass kernel for nn_DWT_Layer: 3-level 2D db4 DWT (symmetric mode).

Input  x: (16, 3, 1024, 1024) fp32.
Output:   (16, 3, 64, 128, 128) fp32 — the flattened/truncated wavelet pyramid
          [cA3, cH3, cV3, cD3, cH2, cV2, cD2, cH1, cV1, cD1(truncated)].

Sharding: pure data parallel — the 48 (batch*channel) images are split 6 per
NeuronCore across 8 cores; no communication.

v2 design (vs fp32 baseline):
  * all intermediates fp16: matmuls run 1 cyc/row (4x over fp32), and the
    width-pass MACs on DVE hit the 4x_2p perf mode (needs 2-byte dtype +
    unit-stride operands in SBUF).
  * width pass is POLYPHASE: the symmetric-extension buffer is split into
    even/odd column buffers so every tap reads unit-stride.  ext_e[u]=ext[2u],
    ext_o[u]=ext[2u+1]; out[c] = sum_k frev[2k]*ext_e[c+k] + frev[2k+1]*ext_o[c+k].
  * height pass: banded fp16 matmuls; the hi filter block sits at a
    128-aligned row offset so psum->staging copies are slot-aligned.
  * detail outputs staged per whole section in fp16 and written with a few
    large DMAs (HWDGE instruction overhead is ~650ns each); host converts the
    fp16 output back to fp32.
"""
import numpy as np

# ----------------------------------------------------------------- constants
DEC_LO = np.array([-0.010597401784997278, 0.032883011666982945,
                   0.030841381835986965, -0.18703481171888114,
                   -0.027983769416983849, 0.63088076792959036,
                   0.71484657055254153, 0.23037781330885523], dtype=np.float64)
L = 8
DEC_HI = np.array([(-1.0) ** (k + 1) * DEC_LO[L - 1 - k] for k in range(L)],
                  dtype=np.float64)
FREV_LO = [float(v) for v in DEC_LO[::-1].astype(np.float32)]
FREV_HI = [float(v) for v in DEC_HI[::-1].astype(np.float32)]
TAPS_ARR = np.tile(np.array(FREV_LO + FREV_HI, dtype=np.float32)[None, :],
                   (128, 1))  # unused; kept for test.py compat

B, C, H, W = 16, 3, 1024, 1024
N_CORES = 8
IMGS_PER_CORE = 6
IMG_ELEMS = H * W

# (N, Np, S_in, n_det_slots)
LEVELS = [
    (1024, 515, 8, 5),
    (515, 261, 5, 3),
    (261, 134, 3, 2),
]

SECT = {}
_cur = 0
for _name, _n in [("cA3", 134), ("cH3", 134), ("cV3", 134), ("cD3", 134),
                  ("cH2", 261), ("cV2", 261), ("cD2", 261),
                  ("cH1", 515), ("cV1", 515), ("cD1", 515)]:
    SECT[_name] = (_cur, _n)
    _cur += _n * _n
CD1_FULL_ROWS = 469
CD1_PART_COLS = 404
assert SECT["cD1"][0] + CD1_FULL_ROWS * 515 + CD1_PART_COLS == IMG_ELEMS


def nprime(N):
    return (N + 5) // 2 + 1


def ext_index(j, N):
    if j < 6:
        return 5 - j
    if j < N + 6:
        return j - 6
    return 2 * N + 5 - j


def dwt_matrix(N, filt):
    Np = nprime(N)
    M = np.zeros((Np, N), dtype=np.float64)
    filtrev = filt[::-1]
    for i in range(Np):
        for t in range(L):
            M[i, ext_index(2 * i + t, N)] += filtrev[t]
    return M


def hi_off(Np):
    """row offset of the hi section: 128-aligned so hi t-tiles map 1:1 to
    128-row detail slots (and engine APs start at partition 0)."""
    return ((Np + 127) // 128) * 128


def stacked_matrix(N):
    Np = nprime(N)
    off = hi_off(Np)
    M2 = np.zeros((off + Np, N), dtype=np.float64)
    M2[0:Np] = dwt_matrix(N, DEC_LO)
    M2[off:] = dwt_matrix(N, DEC_HI)
    return M2


# ---- db4 lifting factorization of the width pass (derived in lifting3.py,
# verified exact to 4e-12 and to 1.1e-3 in fp16).  W0 = even buffer, W1 = odd
# buffer; steps run in order, each: Wtgt[dt:dt+span] += coef*Wsrc[ds:ds+span]
# with span = U - red.  Afterwards lo[c] = C_LO*W1[c], hi[c] = C_HI*W0[c+3];
# both scales are folded into the height-pass matmul weights.
LIFT_STEPS = [  # (tgt, src, coef, dt, ds, red)
    (0, 1, -0.3222758880040146, 0, 0, 0),
    (1, 0, +0.2919531259962464, 0, 0, 0),
    (0, 1, -0.8951560913900637, 1, 0, 1),
    (1, 0, +0.4431871278949297, 0, 1, 1),
    (0, 1, +0.4744486534862916, 2, 0, 2),
    (1, 0, -0.1327810030502859, 0, 2, 2),
    (0, 1, -0.0898286913279579, 3, 0, 3),
    (1, 0, +0.0235063081002452, 0, 3, 3),
]
C_LO = 1.3989015841904142
C_HI = 0.7148465705525415


def band_blocks(N):
    M2 = stacked_matrix(N)
    R = M2.shape[0]
    kt = (N + 127) // 128
    ot = (R + 127) // 128
    per_t = []
    for t in range(ot):
        qs = []
        for q in range(kt):
            blk = M2[t * 128:(t + 1) * 128, q * 128:(q + 1) * 128]
            if np.any(blk != 0):
                qs.append(q)
        per_t.append(qs)
    return per_t, kt, ot, R


def const_weights(N):
    """packed lhsT blocks [128, 2*nblocks, 128] fp16 + index map {(t,q): b}.

    Block b is scaled by C_LO (used when the rhs is the lifted lo channel
    W1); block nb+b is scaled by C_HI (rhs = lifted hi channel W0)."""
    M2 = stacked_matrix(N)
    per_t, kt, ot, R = band_blocks(N)
    blocks = [(t, q) for t in range(ot) for q in per_t[t]]
    nb = len(blocks)
    arr = np.zeros((128, 2 * nb, 128), dtype=np.float16)
    idx = {}
    for b, (t, q) in enumerate(blocks):
        blk = M2[t * 128:(t + 1) * 128, q * 128:(q + 1) * 128]  # [mt, kq]
        arr[:blk.shape[1], b, :blk.shape[0]] = (C_LO * blk.T).astype(
            np.float16)
        arr[:blk.shape[1], nb + b, :blk.shape[0]] = (C_HI * blk.T).astype(
            np.float16)
        idx[(t, q)] = b
    return arr, idx, per_t, nb


WC = {N: const_weights(N) for N, _, _, _ in LEVELS}


# polyphase geometry per level: interior sizes and buffer width
def poly_geom(N):
    Np = nprime(N)
    ne = (N + 1) // 2
    no = N // 2
    U = Np + 3
    return Np, ne, no, U


# mirror copies (dst_buf, dst0, n, src_buf, src_hi) meaning
#   dst[dst0 : dst0+n] = src[src_hi : src_hi-n : -1]
# computed from the generic plan; verified in proto.py against pywt semantics.
def mirror_copies(N):
    Np, ne, no, U = poly_geom(N)

    def src_loc(xi):
        if xi % 2 == 0:
            return 0, xi // 2 + 3
        return 1, (xi - 1) // 2 + 3

    out = []
    for buf, n_int in ((0, ne), (1, no)):
        for rng in (range(0, 3), range(3 + n_int, U)):
            runs = [(u, src_loc(ext_index(2 * u + buf, N))) for u in rng]
            i = 0
            while i < len(runs):
                du0, (sb, su0) = runs[i]
                j = i + 1
                while (j < len(runs) and runs[j][1][0] == sb
                       and runs[j][0] == runs[j - 1][0] + 1
                       and runs[j][1][1] == runs[j - 1][1][1] - 1):
                    j += 1
                out.append((buf, du0, j - i, sb, su0))
                i = j
    return out


# ---- engine assignment knobs ----
DEINT_EVEN_ENG = "scalar"   # fp32->fp16 de-interleave, even phase
DEINT_ODD_ENG = "gpsimd"
MIRROR_ENG = "gpsimd"
AA_ENG = "scalar"           # psum -> next-level ext copies
DET_ENG = {"cH": "scalar", "cV": "gpsimd", "cD": "gpsimd", "cA": "scalar"}
MAC_SLOT_SPLIT = {1: 2, 2: 1, 3: 1}  # ops per tap (split over slots)

XF_BUFS = 2
EXT1_BUFS = 2
WB1_BUFS = 2
DET_BUFS = 2

_BUILT = None


def _eng(nc, name):
    return getattr(nc, name)


def _copy(nc, eng, out, in_):
    """engine-dispatched copy: ACT uses activation-Copy, others tensor_copy."""
    if eng == "scalar":
        nc.scalar.copy(out=out, in_=in_)
    else:
        getattr(nc, eng).tensor_copy(out=out, in_=in_)


def _emit_mirrors(nc, N, ee, eo, S):
    bufs = {0: ee, 1: eo}
    for (db, du0, n, sb, su0) in mirror_copies(N):
        _copy(nc, MIRROR_ENG, bufs[db][:, 0:S, du0:du0 + n],
              bufs[sb][:, 0:S, su0:su0 - n:-1])


def _emit_lift(nc, tmp_pool, lvl, N, ee, eo, S):
    """In-place lifting width pass on the polyphase buffers.

    After the 8 steps, eo holds the (1/C_LO-scaled) lo channel over [0, Np)
    and ee holds the (1/C_HI-scaled) hi channel over [3, Np+3); the height
    matmul reads them directly (scales folded into the weights).  Each step
    is a tensor_scalar mult (4x_2p) into a tmp + tensor_tensor add (2x_1p):
    6n DVE cycles per 16n-elem filter pair vs 11n for direct mult+add."""
    import concourse.mybir as mybir
    Np = nprime(N)
    U = Np + 3
    bufs = {0: ee, 1: eo}
    nsub = MAC_SLOT_SPLIT[lvl]
    bounds = [round(S * i / nsub) for i in range(nsub + 1)]
    f16 = mybir.dt.float16
    for si in range(nsub):
        s0, s1 = bounds[si], bounds[si + 1]
        ns = s1 - s0
        for step_i, (tg, sr, coef, dt_, ds, red) in enumerate(LIFT_STEPS):
            span = U - red
            src = bufs[sr][:, s0:s1, ds:ds + span]
            dst = bufs[tg][:, s0:s1, dt_:dt_ + span]
            tmp = tmp_pool.tile([128, ns, U], f16,
                                tag=f"ltmp{lvl}_{si}", bufs=2,
                                name=f"ltmp{lvl}_{si}_{step_i}")
            nc.vector.tensor_scalar_mul(tmp[:, 0:ns, 0:span], src, coef)
            nc.vector.tensor_tensor(out=dst, in0=dst,
                                    in1=tmp[:, 0:ns, 0:span],
                                    op=mybir.AluOpType.add)


def build_bass(n_images=IMGS_PER_CORE, repeats=1):
    import concourse.mybir as mybir
    import concourse.tile as tile
    from concourse import bacc
    from contextlib import ExitStack

    nc = bacc.Bacc("TRN2", target_bir_lowering=False, debug=False)

    xin = nc.dram_tensor("xin", (n_images, H, W), mybir.dt.float32,
                         kind="ExternalInput").ap()
    out = nc.dram_tensor("out", (n_images, IMG_ELEMS), mybir.dt.float16,
                         kind="ExternalOutput").ap()
    wdram = {}
    for N, _, _, _ in LEVELS:
        arr = WC[N][0]
        wdram[N] = nc.dram_tensor(f"w{N}", arr.shape, mybir.dt.float16,
                                  kind="ExternalInput").ap()

    with tile.TileContext(nc) as tc, ExitStack() as ctx:
        cpool = ctx.enter_context(tc.tile_pool(name="consts", bufs=1))
        extp = ctx.enter_context(tc.tile_pool(name="ext", bufs=1))
        wbp = ctx.enter_context(tc.tile_pool(name="wb", bufs=1))
        psp = ctx.enter_context(tc.tile_pool(name="ps", bufs=1, space="PSUM"))
        detp = ctx.enter_context(tc.tile_pool(name="det", bufs=1))

        wsb = {}
        for N, _, _, _ in LEVELS:
            arr = WC[N][0]
            wsb[N] = cpool.tile(list(arr.shape), mybir.dt.float16,
                                name=f"wsb{N}")
            nc.sync.dma_start(out=wsb[N][:], in_=wdram[N])

        for _rep in range(repeats):
            for img in range(n_images):
                _emit_image(nc, tc, extp, wbp, psp, detp, wsb, xin, out, img)

    nc.compile()
    return nc


def _emit_image(nc, tc, extp, wbp, psp, detp, wsb, xin, out, img):
    import concourse.mybir as mybir
    f16 = mybir.dt.float16

    # ---------------- L1: load + de-interleave + MACs ----------------
    N1, P1 = 1024, 515
    _, ne1, no1, U1 = poly_geom(N1)
    ee1 = extp.tile([128, 8, U1], f16, tag="ext1e", bufs=EXT1_BUFS,
                    name=f"ee1_{img}")
    eo1 = extp.tile([128, 8, U1], f16, tag="ext1o", bufs=EXT1_BUFS,
                    name=f"eo1_{img}")
    for h in range(2):
        xf = extp.tile([128, 4, W], mybir.dt.float32, tag="xf", bufs=XF_BUFS,
                       name=f"xf_{img}_{h}")
        src = xin[img, 512 * h:512 * (h + 1), :].rearrange(
            "(s p) w -> p s w", p=128)
        nc.sync.dma_start(out=xf[:], in_=src)
        _copy(nc, DEINT_EVEN_ENG, ee1[:, 4 * h:4 * h + 4, 3:3 + ne1],
              xf[:, :, 0:W:2])
        _copy(nc, DEINT_ODD_ENG, eo1[:, 4 * h:4 * h + 4, 3:3 + no1],
              xf[:, :, 1:W:2])
    _emit_mirrors(nc, N1, ee1, eo1, 8)

    _emit_lift(nc, wbp, 1, N1, ee1, eo1, 8)

    # next-level aa staging (fp16, straight layout) + polyphase ext buffers.
    # The aa quadrant lands contiguously in aa2/aa3 (one ACT copy per psum
    # tile); Pool then de-interleaves SBUF->SBUF into the ext buffers.
    # Tail slots are memset so unwritten partitions stay finite.
    _, ne2, no2, U2 = poly_geom(515)
    aa2 = extp.tile([128, 5, 515], f16, tag="aa2", bufs=1, name=f"aa2_{img}")
    nc.gpsimd.memset(aa2[:, 4, :], 0.0)
    ee2 = extp.tile([128, 5, U2], f16, tag="ext2e", bufs=1, name=f"ee2_{img}")
    eo2 = extp.tile([128, 5, U2], f16, tag="ext2o", bufs=1, name=f"eo2_{img}")
    _, ne3, no3, U3 = poly_geom(261)
    aa3 = extp.tile([128, 3, 261], f16, tag="aa3", bufs=1, name=f"aa3_{img}")
    nc.gpsimd.memset(aa3[:, 2, :], 0.0)
    ee3 = extp.tile([128, 3, U3], f16, tag="ext3e", bufs=1, name=f"ee3_{img}")
    eo3 = extp.tile([128, 3, U3], f16, tag="ext3o", bufs=1, name=f"eo3_{img}")

    # detail staging tiles (fp16), whole sections, sec-major slots
    det1 = detp.tile([128, 10, 515], f16, tag="det1", bufs=DET_BUFS,
                     name=f"det1_{img}")  # cH1 slots 0-4, cV1 slots 5-9
    cd1 = detp.tile([128, 4, 515], f16, tag="cd1", bufs=DET_BUFS,
                    name=f"cd1_{img}")    # cD1 rows 0..511 (trunc at 469)
    det2 = detp.tile([128, 9, 261], f16, tag="det2", bufs=DET_BUFS,
                     name=f"det2_{img}")  # cH2 0-2, cV2 3-5, cD2 6-8
    det3 = detp.tile([128, 8, 134], f16, tag="det3", bufs=DET_BUFS,
                     name=f"det3_{img}")  # cA3 0-1, cH3 2-3, cV3 4-5, cD3 6-7

    def rhs1(q, qd, c0, c1):
        if qd == 0:
            return eo1[:, q, c0:c1]
        return ee1[:, q, 3 + c0:3 + c1]

    _emit_level_mm(nc, psp, wsb, img, N=1024, rhs=rhs1, aa=aa2,
                   det_cH=(det1, 0), det_cV=(det1, 5), det_cD=(cd1, 0),
                   det_cA=None, cd_trunc=True)
    # Pool de-interleaves aa2 -> polyphase ext (SBUF->SBUF; Pool can't
    # read PSUM), then mirrors
    nc.gpsimd.tensor_copy(out=ee2[:, 0:5, 3:3 + ne2], in_=aa2[:, :, 0:515:2])
    nc.gpsimd.tensor_copy(out=eo2[:, 0:5, 3:3 + no2], in_=aa2[:, :, 1:515:2])
    _emit_mirrors(nc, 515, ee2, eo2, 5)

    _emit_lift(nc, wbp, 2, 515, ee2, eo2, 5)

    def rhs2(q, qd, c0, c1):
        p1 = 3 if q == 4 else 128
        if qd == 0:
            return eo2[0:p1, q, c0:c1]
        return ee2[0:p1, q, 3 + c0:3 + c1]

    _emit_level_mm(nc, psp, wsb, img, N=515, rhs=rhs2, aa=aa3,
                   det_cH=(det2, 0), det_cV=(det2, 3), det_cD=(det2, 6),
                   det_cA=None, cd_trunc=False)
    nc.gpsimd.tensor_copy(out=ee3[:, 0:3, 3:3 + ne3], in_=aa3[:, :, 0:261:2])
    nc.gpsimd.tensor_copy(out=eo3[:, 0:3, 3:3 + no3], in_=aa3[:, :, 1:261:2])
    _emit_mirrors(nc, 261, ee3, eo3, 3)

    _emit_lift(nc, wbp, 3, 261, ee3, eo3, 3)

    def rhs3(q, qd, c0, c1):
        p1 = 5 if q == 2 else 128
        if qd == 0:
            return eo3[0:p1, q, c0:c1]
        return ee3[0:p1, q, 3 + c0:3 + c1]

    _emit_level_mm(nc, psp, wsb, img, N=261, rhs=rhs3,
                   aa=None,
                   det_cH=(det3, 2), det_cV=(det3, 4), det_cD=(det3, 6),
                   det_cA=(det3, 0), cd_trunc=False)

    # ---------------- output DMAs ----------------
    # L1: cH1+cV1 mains (4 full slots each), then a combined 3-row tail
    for sec, name in ((0, "cH1"), (1, "cV1")):
        b = SECT[name][0]
        dst = out[img, b:b + 512 * 515].rearrange("(s p w) -> p s w",
                                                  p=128, w=515)
        nc.sync.dma_start(out=dst, in_=det1[:, 5 * sec:5 * sec + 4, :])
    bh = SECT["cH1"][0]
    # combined tail: rows 512..514 of cH1 and cV1 via sec-strided AP
    dstT = out[img, bh:bh + 2 * 515 * 515].rearrange(
        "(sec p w) -> p sec w", sec=2, w=515)
    nc.sync.dma_start(out=dstT[512:515, :, :], in_=det1[0:3, 4:10:5, :])
    # cD1: 3 full slots, then 85 rows, then the 404-col partial row
    bd = SECT["cD1"][0]
    dst = out[img, bd:bd + 384 * 515].rearrange("(s p w) -> p s w",
                                                p=128, w=515)
    nc.sync.dma_start(out=dst, in_=cd1[:, 0:3, :])
    dst = out[img, bd + 384 * 515:bd + 469 * 515].rearrange(
        "(p w) -> p w", w=515)
    nc.sync.dma_start(out=dst, in_=cd1[0:85, 3, :])
    dst = out[img, bd + 469 * 515:bd + 469 * 515 + 404].rearrange(
        "(p w) -> p w", w=404)
    nc.sync.dma_start(out=dst, in_=cd1[85:86, 3, 0:404])
    # L2: three sections, contiguous: two 128-row passes + 5-row tail
    b2 = SECT["cH2"][0]
    dst2 = out[img, b2:b2 + 3 * 261 * 261].rearrange(
        "(sec p w) -> p sec w", sec=3, w=261)
    nc.sync.dma_start(out=dst2[0:128, :, :], in_=det2[:, 0:9:3, :])
    nc.sync.dma_start(out=dst2[128:256, :, :], in_=det2[:, 1:9:3, :])
    nc.sync.dma_start(out=dst2[256:261, :, :], in_=det2[0:5, 2:9:3, :])
    # L3: four sections, contiguous from offset 0: main + 6-row tail
    dst3 = out[img, 0:4 * 134 * 134].rearrange(
        "(sec p w) -> p sec w", sec=4, w=134)
    nc.sync.dma_start(out=dst3[0:128, :, :], in_=det3[:, 0:8:2, :])
    nc.sync.dma_start(out=dst3[128:134, :, :], in_=det3[0:6, 1:8:2, :])


def _free_chunks(Np):
    out = []
    for base in (0, Np):
        c = 0
        while c < Np:
            e = min(c + 512, Np)
            out.append((base + c, base + e))
            c = e
    return out


def _emit_level_mm(nc, psp, wsb, img, N, rhs, aa, det_cH, det_cV, det_cD,
                   det_cA, cd_trunc):
    """height-pass matmuls + one ACT psum->sbuf copy per (tile, quadrant).

    Each quadrant gets a [128, PSPAD] fp32 psum tile (PSPAD is a multiple of
    512 so every ring buffer stays bank-aligned); matmul column-groups of
    <=512 land in separate banks, and a single ACT copy drains the whole
    quadrant (engine reads may cross banks)."""
    import concourse.mybir as mybir

    Np = nprime(N)
    arr, idx, per_t, nb = WC[N]
    OFF = hi_off(Np)
    R = OFF + Np
    ot = (R + 127) // 128
    lo_tiles = (Np + 127) // 128
    pspad = 1024 if Np > 512 else 512
    pstag = f"ps{pspad}"
    sub = [(c, min(c + 512, Np)) for c in range(0, Np, 512)]

    for t in range(ot):
        is_lo = t < lo_tiles
        slot = t if is_lo else t - OFF // 128
        mrows = min(128, Np - 128 * slot)
        last_hi = (not is_lo) and slot == lo_tiles - 1
        qs = per_t[t]
        if not qs:
            continue
        quadrants = (0,) if (cd_trunc and last_hi) else (0, 1)
        for qd in quadrants:
            ps = psp.tile([128, pspad], mybir.dt.float32, tag=pstag,
                          bufs=3 if pspad == 1024 else 2,
                          name=f"ps_{img}_{N}_{t}_{qd}")
            for (c0, c1) in sub:
                w = c1 - c0
                for ki, q in enumerate(qs):
                    kq = min(128, N - q * 128)
                    r = rhs(q, qd, c0, c1)
                    wblk = idx[(t, q)] + (nb if qd == 1 else 0)
                    nc.tensor.matmul(
                        ps[0:mrows, c0:c1],
                        wsb[N][0:kq, wblk, 0:mrows],
                        r,
                        start=(ki == 0), stop=(ki == len(qs) - 1))
            # single drain copy for the whole quadrant
            if is_lo:
                if qd == 0 and aa is not None:
                    nc.scalar.copy(out=aa[0:mrows, slot, 0:Np],
                                   in_=ps[0:mrows, 0:Np])
                elif qd == 0:
                    dt_tile, s0 = det_cA
                    nc.scalar.copy(out=dt_tile[0:mrows, s0 + slot, 0:Np],
                                   in_=ps[0:mrows, 0:Np])
                else:
                    dt_tile, s0 = det_cV
                    nc.scalar.copy(out=dt_tile[0:mrows, s0 + slot, 0:Np],
                                   in_=ps[0:mrows, 0:Np])
            else:
                if qd == 0:
                    dt_tile, s0 = det_cH
                    nc.scalar.copy(out=dt_tile[0:mrows, s0 + slot, 0:Np],
                                   in_=ps[0:mrows, 0:Np])
                else:
                    dt_tile, s0 = det_cD
                    nc.scalar.copy(out=dt_tile[0:mrows, s0 + slot, 0:Np],
                                   in_=ps[0:mrows, 0:Np])


# ----------------------------------------------------------------- runner
def _get_built():
    global _BUILT
    if _BUILT is None:
        _BUILT = build_bass()
    return _BUILT


def kernel(x: np.ndarray) -> np.ndarray:
    from concourse import bass_utils

    x = np.ascontiguousarray(np.asarray(x), dtype=np.float32)
    assert x.shape == (B, C, H, W), x.shape
    nc = _get_built()

    imgs = x.reshape(B * C, H, W)
    in_maps = []
    for c in range(N_CORES):
        m = {"xin": imgs[c * IMGS_PER_CORE:(c + 1) * IMGS_PER_CORE]}
        for N, _, _, _ in LEVELS:
            m[f"w{N}"] = WC[N][0]
        in_maps.append(m)

    res = bass_utils.run_bass_kernel_spmd(nc, in_maps,
                                          core_ids=list(range(N_CORES)))
    outs = [res.results[c]["out"] for c in range(N_CORES)]
    flat = np.concatenate(outs, axis=0)  # [48, 1048576] fp16
    return flat.astype(np.float32).reshape(B, C, 64, 128, 128)


# revision 24
# speedup vs baseline: 3.9598x; 1.7059x over previous
"""Trainium2 Bass kernel for nn_DWT_Layer: 3-level 2D db4 DWT (symmetric mode).

Input  x: (16, 3, 1024, 1024) fp32.
Output:   (16, 3, 64, 128, 128) fp32 — the flattened/truncated wavelet pyramid
          [cA3, cH3, cV3, cD3, cH2, cV2, cD2, cH1, cV1, cD1(truncated)].

Sharding: pure data parallel — the 48 (batch*channel) images are split 6 per
NeuronCore across 8 cores; no communication.

v2 design (vs fp32 baseline):
  * all intermediates fp16: matmuls run 1 cyc/row (4x over fp32), and the
    width-pass MACs on DVE hit the 4x_2p perf mode (needs 2-byte dtype +
    unit-stride operands in SBUF).
  * width pass is POLYPHASE: the symmetric-extension buffer is split into
    even/odd column buffers so every tap reads unit-stride.  ext_e[u]=ext[2u],
    ext_o[u]=ext[2u+1]; out[c] = sum_k frev[2k]*ext_e[c+k] + frev[2k+1]*ext_o[c+k].
  * height pass: banded fp16 matmuls; the hi filter block sits at a
    128-aligned row offset so psum->staging copies are slot-aligned.
  * detail outputs staged per whole section in fp16 and written with a few
    large DMAs (HWDGE instruction overhead is ~650ns each); host converts the
    fp16 output back to fp32.
"""
import numpy as np

# ----------------------------------------------------------------- constants
DEC_LO = np.array([-0.010597401784997278, 0.032883011666982945,
                   0.030841381835986965, -0.18703481171888114,
                   -0.027983769416983849, 0.63088076792959036,
                   0.71484657055254153, 0.23037781330885523], dtype=np.float64)
L = 8
DEC_HI = np.array([(-1.0) ** (k + 1) * DEC_LO[L - 1 - k] for k in range(L)],
                  dtype=np.float64)
FREV_LO = [float(v) for v in DEC_LO[::-1].astype(np.float32)]
FREV_HI = [float(v) for v in DEC_HI[::-1].astype(np.float32)]
TAPS_ARR = np.tile(np.array(FREV_LO + FREV_HI, dtype=np.float32)[None, :],
                   (128, 1))  # unused; kept for test.py compat

B, C, H, W = 16, 3, 1024, 1024
N_CORES = 8
IMGS_PER_CORE = 6
IMG_ELEMS = H * W

# (N, Np, S_in, n_det_slots)
LEVELS = [
    (1024, 515, 8, 5),
    (515, 261, 5, 3),
    (261, 134, 3, 2),
]

SECT = {}
_cur = 0
for _name, _n in [("cA3", 134), ("cH3", 134), ("cV3", 134), ("cD3", 134),
                  ("cH2", 261), ("cV2", 261), ("cD2", 261),
                  ("cH1", 515), ("cV1", 515), ("cD1", 515)]:
    SECT[_name] = (_cur, _n)
    _cur += _n * _n
CD1_FULL_ROWS = 469
CD1_PART_COLS = 404
assert SECT["cD1"][0] + CD1_FULL_ROWS * 515 + CD1_PART_COLS == IMG_ELEMS


def nprime(N):
    return (N + 5) // 2 + 1


def ext_index(j, N):
    if j < 6:
        return 5 - j
    if j < N + 6:
        return j - 6
    return 2 * N + 5 - j


def dwt_matrix(N, filt):
    Np = nprime(N)
    M = np.zeros((Np, N), dtype=np.float64)
    filtrev = filt[::-1]
    for i in range(Np):
        for t in range(L):
            M[i, ext_index(2 * i + t, N)] += filtrev[t]
    return M


def hi_off(Np):
    """row offset of the hi section: 128-aligned so hi t-tiles map 1:1 to
    128-row detail slots (and engine APs start at partition 0)."""
    return ((Np + 127) // 128) * 128


def stacked_matrix(N):
    Np = nprime(N)
    off = hi_off(Np)
    M2 = np.zeros((off + Np, N), dtype=np.float64)
    M2[0:Np] = dwt_matrix(N, DEC_LO)
    M2[off:] = dwt_matrix(N, DEC_HI)
    return M2


# ---- db4 lifting factorization of the width pass (derived in lifting3.py,
# verified exact to 4e-12 and to 1.1e-3 in fp16).  W0 = even buffer, W1 = odd
# buffer; steps run in order, each: Wtgt[dt:dt+span] += coef*Wsrc[ds:ds+span]
# with span = U - red.  Afterwards lo[c] = C_LO*W1[c], hi[c] = C_HI*W0[c+3];
# both scales are folded into the height-pass matmul weights.
LIFT_STEPS = [  # (tgt, src, coef, dt, ds, red)
    (0, 1, -0.3222758880040146, 0, 0, 0),
    (1, 0, +0.2919531259962464, 0, 0, 0),
    (0, 1, -0.8951560913900637, 1, 0, 1),
    (1, 0, +0.4431871278949297, 0, 1, 1),
    (0, 1, +0.4744486534862916, 2, 0, 2),
    (1, 0, -0.1327810030502859, 0, 2, 2),
    (0, 1, -0.0898286913279579, 3, 0, 3),
    (1, 0, +0.0235063081002452, 0, 3, 3),
]
C_LO = 1.3989015841904142
C_HI = 0.7148465705525415


def band_blocks(N):
    M2 = stacked_matrix(N)
    R = M2.shape[0]
    kt = (N + 127) // 128
    ot = (R + 127) // 128
    per_t = []
    for t in range(ot):
        qs = []
        for q in range(kt):
            blk = M2[t * 128:(t + 1) * 128, q * 128:(q + 1) * 128]
            if np.any(blk != 0):
                qs.append(q)
        per_t.append(qs)
    return per_t, kt, ot, R


def const_weights(N):
    """packed lhsT blocks [128, 2*nblocks, 128] fp16 + index map {(t,q): b}.

    Block b is scaled by C_LO (used when the rhs is the lifted lo channel
    W1); block nb+b is scaled by C_HI (rhs = lifted hi channel W0)."""
    M2 = stacked_matrix(N)
    per_t, kt, ot, R = band_blocks(N)
    blocks = [(t, q) for t in range(ot) for q in per_t[t]]
    nb = len(blocks)
    arr = np.zeros((128, 2 * nb, 128), dtype=np.float16)
    idx = {}
    for b, (t, q) in enumerate(blocks):
        blk = M2[t * 128:(t + 1) * 128, q * 128:(q + 1) * 128]  # [mt, kq]
        arr[:blk.shape[1], b, :blk.shape[0]] = (C_LO * blk.T).astype(
            np.float16)
        arr[:blk.shape[1], nb + b, :blk.shape[0]] = (C_HI * blk.T).astype(
            np.float16)
        idx[(t, q)] = b
    return arr, idx, per_t, nb


WC = {N: const_weights(N) for N, _, _, _ in LEVELS}


# polyphase geometry per level: interior sizes and buffer width
def poly_geom(N):
    Np = nprime(N)
    ne = (N + 1) // 2
    no = N // 2
    U = Np + 3
    return Np, ne, no, U


# mirror copies (dst_buf, dst0, n, src_buf, src_hi) meaning
#   dst[dst0 : dst0+n] = src[src_hi : src_hi-n : -1]
# computed from the generic plan; verified in proto.py against pywt semantics.
def mirror_copies(N):
    Np, ne, no, U = poly_geom(N)

    def src_loc(xi):
        if xi % 2 == 0:
            return 0, xi // 2 + 3
        return 1, (xi - 1) // 2 + 3

    out = []
    for buf, n_int in ((0, ne), (1, no)):
        for rng in (range(0, 3), range(3 + n_int, U)):
            runs = [(u, src_loc(ext_index(2 * u + buf, N))) for u in rng]
            i = 0
            while i < len(runs):
                du0, (sb, su0) = runs[i]
                j = i + 1
                while (j < len(runs) and runs[j][1][0] == sb
                       and runs[j][0] == runs[j - 1][0] + 1
                       and runs[j][1][1] == runs[j - 1][1][1] - 1):
                    j += 1
                out.append((buf, du0, j - i, sb, su0))
                i = j
    return out


# ---- engine assignment knobs ----
DEINT_EVEN_ENG = "scalar"   # fp32->fp16 de-interleave, even phase
DEINT_ODD_ENG = "gpsimd"
MIRROR_ENG = "gpsimd"
AA_ENG = "scalar"           # psum -> next-level ext copies
DET_ENG = {"cH": "scalar", "cV": "gpsimd", "cD": "gpsimd", "cA": "scalar"}
MAC_SLOT_SPLIT = {1: 2, 2: 1, 3: 1}  # ops per tap (split over slots)

XF_BUFS = 2
EXT1_BUFS = 2
WB1_BUFS = 2
DET_BUFS = 2

_BUILT = None


def _eng(nc, name):
    return getattr(nc, name)


def _copy(nc, eng, out, in_):
    """engine-dispatched copy: ACT uses activation-Copy, others tensor_copy."""
    if eng == "scalar":
        nc.scalar.copy(out=out, in_=in_)
    else:
        getattr(nc, eng).tensor_copy(out=out, in_=in_)


def _emit_mirrors(nc, N, ee, eo, S):
    bufs = {0: ee, 1: eo}
    for (db, du0, n, sb, su0) in mirror_copies(N):
        _copy(nc, MIRROR_ENG, bufs[db][:, 0:S, du0:du0 + n],
              bufs[sb][:, 0:S, su0:su0 - n:-1])


def _emit_lift(nc, tmp_pool, lvl, N, ee, eo, S):
    """In-place lifting width pass on the polyphase buffers.

    After the 8 steps, eo holds the (1/C_LO-scaled) lo channel over [0, Np)
    and ee holds the (1/C_HI-scaled) hi channel over [3, Np+3); the height
    matmul reads them directly (scales folded into the weights).  Each step
    is a tensor_scalar mult (4x_2p) into a tmp + tensor_tensor add (2x_1p):
    6n DVE cycles per 16n-elem filter pair vs 11n for direct mult+add."""
    import concourse.mybir as mybir
    Np = nprime(N)
    U = Np + 3
    bufs = {0: ee, 1: eo}
    nsub = MAC_SLOT_SPLIT[lvl]
    bounds = [round(S * i / nsub) for i in range(nsub + 1)]
    f16 = mybir.dt.float16
    # Emit STEP-MAJOR across the slot-group chains: DVE executes in order and
    # each step depends on the previous one (alternating targets), so the
    # write-ack latency between dependent ops is hidden only if independent
    # ops from the sibling chain sit between them in the queue.
    for step_i, (tg, sr, coef, dt_, ds, red) in enumerate(LIFT_STEPS):
        span = U - red
        tmps = []
        for si in range(nsub):
            s0, s1 = bounds[si], bounds[si + 1]
            ns = s1 - s0
            src = bufs[sr][:, s0:s1, ds:ds + span]
            tmp = tmp_pool.tile([128, ns, U], f16,
                                tag=f"ltmp{lvl}_{si}", bufs=2,
                                name=f"ltmp{lvl}_{si}_{step_i}")
            nc.vector.tensor_scalar_mul(tmp[:, 0:ns, 0:span], src, coef)
            tmps.append(tmp)
        for si in range(nsub):
            s0, s1 = bounds[si], bounds[si + 1]
            ns = s1 - s0
            dst = bufs[tg][:, s0:s1, dt_:dt_ + span]
            nc.vector.tensor_tensor(out=dst, in0=dst,
                                    in1=tmps[si][:, 0:ns, 0:span],
                                    op=mybir.AluOpType.add)


def build_bass(n_images=IMGS_PER_CORE, repeats=1):
    import concourse.mybir as mybir
    import concourse.tile as tile
    from concourse import bacc
    from contextlib import ExitStack

    nc = bacc.Bacc("TRN2", target_bir_lowering=False, debug=False)

    xin = nc.dram_tensor("xin", (n_images, H, W), mybir.dt.float32,
                         kind="ExternalInput").ap()
    out = nc.dram_tensor("out", (n_images, IMG_ELEMS), mybir.dt.float16,
                         kind="ExternalOutput").ap()
    wdram = {}
    for N, _, _, _ in LEVELS:
        arr = WC[N][0]
        wdram[N] = nc.dram_tensor(f"w{N}", arr.shape, mybir.dt.float16,
                                  kind="ExternalInput").ap()

    with tile.TileContext(nc) as tc, ExitStack() as ctx:
        cpool = ctx.enter_context(tc.tile_pool(name="consts", bufs=1))
        extp = ctx.enter_context(tc.tile_pool(name="ext", bufs=1))
        wbp = ctx.enter_context(tc.tile_pool(name="wb", bufs=1))
        psp = ctx.enter_context(tc.tile_pool(name="ps", bufs=1, space="PSUM"))
        detp = ctx.enter_context(tc.tile_pool(name="det", bufs=1))

        wsb = {}
        for N, _, _, _ in LEVELS:
            arr = WC[N][0]
            wsb[N] = cpool.tile(list(arr.shape), mybir.dt.float16,
                                name=f"wsb{N}")
            nc.sync.dma_start(out=wsb[N][:], in_=wdram[N])

        for _rep in range(repeats):
            # software-pipelined emission: image i+1's load/deint/L1 stage is
            # emitted BEFORE image i's L2/L3 cascade so the in-order engine
            # queues never convoy the next image's prep behind this image's
            # tail work (that convoy showed up as ~6us DVE gaps per image).
            prev = None
            for img in range(n_images):
                st = _emit_stage_a(nc, extp, wbp, psp, detp, wsb, xin, img)
                if prev is not None:
                    _emit_stage_b(nc, extp, wbp, psp, detp, wsb, out, prev)
                prev = st
            _emit_stage_b(nc, extp, wbp, psp, detp, wsb, out, prev)

    nc.compile()
    return nc


def _emit_stage_a(nc, extp, wbp, psp, detp, wsb, xin, img):
    """Image stage A: input load, de-interleave+convert, mirrors, L1 lifting,
    L1 height matmuls (fills aa2 + L1 detail staging).  Returns the state
    stage B needs."""
    import concourse.mybir as mybir
    f16 = mybir.dt.float16

    # ---------------- L1: load + de-interleave + MACs ----------------
    N1, P1 = 1024, 515
    _, ne1, no1, U1 = poly_geom(N1)
    ee1 = extp.tile([128, 8, U1], f16, tag="ext1e", bufs=EXT1_BUFS,
                    name=f"ee1_{img}")
    eo1 = extp.tile([128, 8, U1], f16, tag="ext1o", bufs=EXT1_BUFS,
                    name=f"eo1_{img}")
    for h in range(2):
        xf = extp.tile([128, 4, W], mybir.dt.float32, tag="xf", bufs=XF_BUFS,
                       name=f"xf_{img}_{h}")
        src = xin[img, 512 * h:512 * (h + 1), :].rearrange(
            "(s p) w -> p s w", p=128)
        nc.sync.dma_start(out=xf[:], in_=src)
        _copy(nc, DEINT_EVEN_ENG, ee1[:, 4 * h:4 * h + 4, 3:3 + ne1],
              xf[:, :, 0:W:2])
        _copy(nc, DEINT_ODD_ENG, eo1[:, 4 * h:4 * h + 4, 3:3 + no1],
              xf[:, :, 1:W:2])
    _emit_mirrors(nc, N1, ee1, eo1, 8)

    _emit_lift(nc, wbp, 1, N1, ee1, eo1, 8)

    # next-level aa staging (fp16, straight layout) + polyphase ext buffers.
    # The aa quadrant lands contiguously in aa2/aa3 (one ACT copy per psum
    # tile); Pool then de-interleaves SBUF->SBUF into the ext buffers.
    # Tail slots are memset so unwritten partitions stay finite.
    _, ne2, no2, U2 = poly_geom(515)
    aa2 = extp.tile([128, 5, 515], f16, tag="aa2", bufs=2, name=f"aa2_{img}")
    nc.gpsimd.memset(aa2[:, 4, :], 0.0)
    ee2 = extp.tile([128, 5, U2], f16, tag="ext2e", bufs=2, name=f"ee2_{img}")
    eo2 = extp.tile([128, 5, U2], f16, tag="ext2o", bufs=2, name=f"eo2_{img}")
    _, ne3, no3, U3 = poly_geom(261)
    aa3 = extp.tile([128, 3, 261], f16, tag="aa3", bufs=2, name=f"aa3_{img}")
    nc.gpsimd.memset(aa3[:, 2, :], 0.0)
    ee3 = extp.tile([128, 3, U3], f16, tag="ext3e", bufs=2, name=f"ee3_{img}")
    eo3 = extp.tile([128, 3, U3], f16, tag="ext3o", bufs=2, name=f"eo3_{img}")

    # detail staging tiles (fp16), whole sections, sec-major slots
    det1 = detp.tile([128, 10, 515], f16, tag="det1", bufs=DET_BUFS,
                     name=f"det1_{img}")  # cH1 slots 0-4, cV1 slots 5-9
    cd1 = detp.tile([128, 4, 515], f16, tag="cd1", bufs=DET_BUFS,
                    name=f"cd1_{img}")    # cD1 rows 0..511 (trunc at 469)
    det2 = detp.tile([128, 9, 261], f16, tag="det2", bufs=DET_BUFS,
                     name=f"det2_{img}")  # cH2 0-2, cV2 3-5, cD2 6-8
    det3 = detp.tile([128, 8, 134], f16, tag="det3", bufs=DET_BUFS,
                     name=f"det3_{img}")  # cA3 0-1, cH3 2-3, cV3 4-5, cD3 6-7

    def rhs1(q, qd, c0, c1):
        if qd == 0:
            return eo1[:, q, c0:c1]
        return ee1[:, q, 3 + c0:3 + c1]

    _emit_level_mm(nc, psp, wsb, img, N=1024, rhs=rhs1, aa=aa2,
                   det_cH=(det1, 0), det_cV=(det1, 5), det_cD=(cd1, 0),
                   det_cA=None, cd_trunc=True)
    return dict(img=img, aa2=aa2, aa3=aa3, ee2=ee2, eo2=eo2, ee3=ee3,
                eo3=eo3, det1=det1, cd1=cd1, det2=det2, det3=det3,
                ne2=ne2, no2=no2, ne3=ne3, no3=no3)


def _emit_stage_b(nc, extp, wbp, psp, detp, wsb, out, st):
    """Image stage B: L2/L3 cascade + all output DMAs."""
    import concourse.mybir as mybir
    img = st["img"]
    aa2, aa3 = st["aa2"], st["aa3"]
    ee2, eo2, ee3, eo3 = st["ee2"], st["eo2"], st["ee3"], st["eo3"]
    det1, cd1, det2, det3 = st["det1"], st["cd1"], st["det2"], st["det3"]
    ne2, no2, ne3, no3 = st["ne2"], st["no2"], st["ne3"], st["no3"]

    # Pool de-interleaves aa2 -> polyphase ext (SBUF->SBUF; Pool can't
    # read PSUM), then mirrors
    nc.gpsimd.tensor_copy(out=ee2[:, 0:5, 3:3 + ne2], in_=aa2[:, :, 0:515:2])
    nc.gpsimd.tensor_copy(out=eo2[:, 0:5, 3:3 + no2], in_=aa2[:, :, 1:515:2])
    _emit_mirrors(nc, 515, ee2, eo2, 5)

    _emit_lift(nc, wbp, 2, 515, ee2, eo2, 5)

    def rhs2(q, qd, c0, c1):
        p1 = 3 if q == 4 else 128
        if qd == 0:
            return eo2[0:p1, q, c0:c1]
        return ee2[0:p1, q, 3 + c0:3 + c1]

    _emit_level_mm(nc, psp, wsb, img, N=515, rhs=rhs2, aa=aa3,
                   det_cH=(det2, 0), det_cV=(det2, 3), det_cD=(det2, 6),
                   det_cA=None, cd_trunc=False)
    nc.gpsimd.tensor_copy(out=ee3[:, 0:3, 3:3 + ne3], in_=aa3[:, :, 0:261:2])
    nc.gpsimd.tensor_copy(out=eo3[:, 0:3, 3:3 + no3], in_=aa3[:, :, 1:261:2])
    _emit_mirrors(nc, 261, ee3, eo3, 3)

    _emit_lift(nc, wbp, 3, 261, ee3, eo3, 3)

    def rhs3(q, qd, c0, c1):
        p1 = 5 if q == 2 else 128
        if qd == 0:
            return eo3[0:p1, q, c0:c1]
        return ee3[0:p1, q, 3 + c0:3 + c1]

    _emit_level_mm(nc, psp, wsb, img, N=261, rhs=rhs3,
                   aa=None,
                   det_cH=(det3, 2), det_cV=(det3, 4), det_cD=(det3, 6),
                   det_cA=(det3, 0), cd_trunc=False)

    # ---------------- output DMAs ----------------
    # L1: cH1+cV1 mains (4 full slots each), then a combined 3-row tail
    for sec, name in ((0, "cH1"), (1, "cV1")):
        b = SECT[name][0]
        dst = out[img, b:b + 512 * 515].rearrange("(s p w) -> p s w",
                                                  p=128, w=515)
        nc.sync.dma_start(out=dst, in_=det1[:, 5 * sec:5 * sec + 4, :])
    bh = SECT["cH1"][0]
    # combined tail: rows 512..514 of cH1 and cV1 via sec-strided AP
    dstT = out[img, bh:bh + 2 * 515 * 515].rearrange(
        "(sec p w) -> p sec w", sec=2, w=515)
    nc.sync.dma_start(out=dstT[512:515, :, :], in_=det1[0:3, 4:10:5, :])
    # cD1: 3 full slots, then 85 rows, then the 404-col partial row
    bd = SECT["cD1"][0]
    dst = out[img, bd:bd + 384 * 515].rearrange("(s p w) -> p s w",
                                                p=128, w=515)
    nc.sync.dma_start(out=dst, in_=cd1[:, 0:3, :])
    dst = out[img, bd + 384 * 515:bd + 469 * 515].rearrange(
        "(p w) -> p w", w=515)
    nc.sync.dma_start(out=dst, in_=cd1[0:85, 3, :])
    dst = out[img, bd + 469 * 515:bd + 469 * 515 + 404].rearrange(
        "(p w) -> p w", w=404)
    nc.sync.dma_start(out=dst, in_=cd1[85:86, 3, 0:404])
    # L2: three sections, contiguous: two 128-row passes + 5-row tail
    b2 = SECT["cH2"][0]
    dst2 = out[img, b2:b2 + 3 * 261 * 261].rearrange(
        "(sec p w) -> p sec w", sec=3, w=261)
    nc.sync.dma_start(out=dst2[0:128, :, :], in_=det2[:, 0:9:3, :])
    nc.sync.dma_start(out=dst2[128:256, :, :], in_=det2[:, 1:9:3, :])
    nc.sync.dma_start(out=dst2[256:261, :, :], in_=det2[0:5, 2:9:3, :])
    # L3: four sections, contiguous from offset 0: main + 6-row tail
    dst3 = out[img, 0:4 * 134 * 134].rearrange(
        "(sec p w) -> p sec w", sec=4, w=134)
    nc.sync.dma_start(out=dst3[0:128, :, :], in_=det3[:, 0:8:2, :])
    nc.sync.dma_start(out=dst3[128:134, :, :], in_=det3[0:6, 1:8:2, :])


def _free_chunks(Np):
    out = []
    for base in (0, Np):
        c = 0
        while c < Np:
            e = min(c + 512, Np)
            out.append((base + c, base + e))
            c = e
    return out


def _emit_level_mm(nc, psp, wsb, img, N, rhs, aa, det_cH, det_cV, det_cD,
                   det_cA, cd_trunc):
    """height-pass matmuls + one ACT psum->sbuf copy per (tile, quadrant).

    Each quadrant gets a [128, PSPAD] fp32 psum tile (PSPAD is a multiple of
    512 so every ring buffer stays bank-aligned); matmul column-groups of
    <=512 land in separate banks, and a single ACT copy drains the whole
    quadrant (engine reads may cross banks)."""
    import concourse.mybir as mybir

    Np = nprime(N)
    arr, idx, per_t, nb = WC[N]
    OFF = hi_off(Np)
    R = OFF + Np
    ot = (R + 127) // 128
    lo_tiles = (Np + 127) // 128
    pspad = 1024 if Np > 512 else 512
    pstag = f"ps{pspad}"
    sub = [(c, min(c + 512, Np)) for c in range(0, Np, 512)]

    for t in range(ot):
        is_lo = t < lo_tiles
        slot = t if is_lo else t - OFF // 128
        mrows = min(128, Np - 128 * slot)
        last_hi = (not is_lo) and slot == lo_tiles - 1
        qs = per_t[t]
        if not qs:
            continue
        quadrants = (0,) if (cd_trunc and last_hi) else (0, 1)
        for qd in quadrants:
            ps = psp.tile([128, pspad], mybir.dt.float32, tag=pstag,
                          bufs=3 if pspad == 1024 else 2,
                          name=f"ps_{img}_{N}_{t}_{qd}")
            for (c0, c1) in sub:
                w = c1 - c0
                for ki, q in enumerate(qs):
                    kq = min(128, N - q * 128)
                    r = rhs(q, qd, c0, c1)
                    wblk = idx[(t, q)] + (nb if qd == 1 else 0)
                    nc.tensor.matmul(
                        ps[0:mrows, c0:c1],
                        wsb[N][0:kq, wblk, 0:mrows],
                        r,
                        start=(ki == 0), stop=(ki == len(qs) - 1))
            # single drain copy for the whole quadrant
            if is_lo:
                if qd == 0 and aa is not None:
                    nc.scalar.copy(out=aa[0:mrows, slot, 0:Np],
                                   in_=ps[0:mrows, 0:Np])
                elif qd == 0:
                    dt_tile, s0 = det_cA
                    nc.scalar.copy(out=dt_tile[0:mrows, s0 + slot, 0:Np],
                                   in_=ps[0:mrows, 0:Np])
                else:
                    dt_tile, s0 = det_cV
                    nc.scalar.copy(out=dt_tile[0:mrows, s0 + slot, 0:Np],
                                   in_=ps[0:mrows, 0:Np])
            else:
                if qd == 0:
                    dt_tile, s0 = det_cH
                    nc.scalar.copy(out=dt_tile[0:mrows, s0 + slot, 0:Np],
                                   in_=ps[0:mrows, 0:Np])
                else:
                    dt_tile, s0 = det_cD
                    nc.scalar.copy(out=dt_tile[0:mrows, s0 + slot, 0:Np],
                                   in_=ps[0:mrows, 0:Np])


# ----------------------------------------------------------------- runner
def _get_built():
    global _BUILT
    if _BUILT is None:
        _BUILT = build_bass()
    return _BUILT


def kernel(x: np.ndarray) -> np.ndarray:
    from concourse import bass_utils

    x = np.ascontiguousarray(np.asarray(x), dtype=np.float32)
    assert x.shape == (B, C, H, W), x.shape
    nc = _get_built()

    imgs = x.reshape(B * C, H, W)
    in_maps = []
    for c in range(N_CORES):
        m = {"xin": imgs[c * IMGS_PER_CORE:(c + 1) * IMGS_PER_CORE]}
        for N, _, _, _ in LEVELS:
            m[f"w{N}"] = WC[N][0]
        in_maps.append(m)

    res = bass_utils.run_bass_kernel_spmd(nc, in_maps,
                                          core_ids=list(range(N_CORES)))
    outs = [res.results[c]["out"] for c in range(N_CORES)]
    flat = np.concatenate(outs, axis=0)  # [48, 1048576] fp16
    return flat.astype(np.float32).reshape(B, C, 64, 128, 128)
